# revision 38
# baseline (speedup 1.0000x reference)
"""Distributed Trainium2 kernel for causal GQA attention with RoPE.

Model: B=2, S=2048, DM=2048, H=16 q-heads, HK=4 kv-heads, D=128.
Sharding over 8 NeuronCores: core c = (batch b=c//4, kv-head kh=c%4).
Each core computes its 4 q-heads / 1 kv-head of one batch end-to-end,
AllGathers attention outputs within its 4-core batch group (split into
head-pair collectives for overlap), and applies a column slice of Wo,
producing out[b][:, kh*512:(kh+1)*512].
"""
import contextlib
import ctypes
import os
import sys
import types

for _p in ("/opt/trn_rl_repo", "/root/.axon_site/_ro/trn_rl_repo"):
    if os.path.isdir(_p) and _p not in sys.path:
        sys.path.insert(0, _p)

import numpy as np
import ml_dtypes

import concourse.bass as bass
import concourse.mybir as mybir
import concourse.tile as tile
from concourse import bacc
from concourse.bass import ts, ds
from concourse.bass_utils import run_bass_kernel_spmd
from concourse.masks import make_identity

BF16 = ml_dtypes.bfloat16
F32 = mybir.dt.float32
BF = mybir.dt.bfloat16

B, S, DM = 2, 2048, 2048
H, HK, D = 16, 4, 128
G = H // HK          # q heads per kv head (= heads per core)
THETA = 10000.0
N_CORES = 8
KT = DM // 128       # 16 K-tiles of the model dim
TOKB = S // 128      # 16 token blocks
TCH = S // 512       # 4 token chunks of 512
HD_CORE = G * D      # 512 output dims of q per core

LAST_EXEC_TIME_NS = None
LAST_RESULTS = None


# ---------------------------------------------------------------- tracing
def _install_ntff_hook():
    """Make run_bass_kernel_spmd(trace=True) work in this container."""
    try:
        from antenv.axon_hooks import get_axon_ntff_profile_hook  # noqa: F401
        return True
    except ImportError:
        pass
    so_path = "/opt/axon/libaxon_pjrt.so"
    if not os.path.exists(so_path):
        return False
    lib = ctypes.CDLL(so_path)
    if not hasattr(lib, "axon_start_nrt_profile"):
        return False
    lib.axon_start_nrt_profile.argtypes = [ctypes.POINTER(ctypes.c_int64), ctypes.c_size_t]
    lib.axon_start_nrt_profile.restype = ctypes.c_int64
    lib.axon_stop_nrt_profile.argtypes = [ctypes.c_char_p]
    lib.axon_stop_nrt_profile.restype = ctypes.c_int64

    @contextlib.contextmanager
    def _hook(output_dir, device_ids):
        import jax
        jax.devices()
        if device_ids:
            ids = (ctypes.c_int64 * len(device_ids))(*device_ids)
            rc = lib.axon_start_nrt_profile(ids, len(device_ids))
        else:
            rc = lib.axon_start_nrt_profile(None, 0)
        if rc != 0:
            raise RuntimeError(f"axon_start_nrt_profile rc={rc}")
        try:
            yield
        finally:
            n = lib.axon_stop_nrt_profile(str(output_dir).encode())
            print(f"profile: {n} file(s) in {output_dir}", file=sys.stderr)

    mod = types.ModuleType("antenv.axon_hooks")
    holder = {"h": _hook}
    mod.set_axon_ntff_profile_hook = lambda h: holder.__setitem__("h", h)
    mod.get_axon_ntff_profile_hook = lambda: holder.get("h")
    sys.modules["antenv.axon_hooks"] = mod
    import antenv
    antenv.axon_hooks = mod
    import concourse.bass_utils as bu
    bu.upload_artifacts = lambda tmpdir: str(tmpdir)
    return True


# ---------------------------------------------------------------- graph
def build_nc():
    nc = bacc.Bacc("TRN2", target_bir_lowering=False, debug=False,
                   num_devices=N_CORES)

    xt = nc.dram_tensor("xt", [DM, S], BF, kind="ExternalInput").ap()
    wq = nc.dram_tensor("wq", [DM, HD_CORE], BF, kind="ExternalInput").ap()
    wk = nc.dram_tensor("wk", [DM, D], BF, kind="ExternalInput").ap()
    wv = nc.dram_tensor("wv", [DM, D], BF, kind="ExternalInput").ap()
    wo = nc.dram_tensor("wo", [DM, HD_CORE], BF, kind="ExternalInput").ap()
    cosq = nc.dram_tensor("cosq", [D, S], BF, kind="ExternalInput").ap()
    sinq = nc.dram_tensor("sinq", [D, S], BF, kind="ExternalInput").ap()
    cosk = nc.dram_tensor("cosk", [D, S], BF, kind="ExternalInput").ap()
    sink = nc.dram_tensor("sink", [D, S], BF, kind="ExternalInput").ap()
    out = nc.dram_tensor("out", [S, HD_CORE], F32, kind="ExternalOutput").ap()

    # batched-DMA views (partition-major over the 128-row K-tiles)
    xt_v = xt.rearrange("(kt p) (c s) -> p kt c s", p=128, c=TCH)
    wq_v = wq.rearrange("(kt p) m -> p kt m", p=128)
    wk_v = wk.rearrange("(kt p) m -> p kt m", p=128)
    wv_v = wv.rearrange("(kt p) m -> p kt m", p=128)
    wo_v = wo.rearrange("(kt p) m -> p kt m", p=128)
    out_v = out.rearrange("(t tb p) c -> p t tb c", tb=4, p=128)

    groups = [[0, 1, 2, 3], [4, 5, 6, 7]]

    with tile.TileContext(nc) as tc:
        with tc.tile_pool(name="const", bufs=1) as cpool, \
             tc.tile_pool(name="wts", bufs=1) as wpool, \
             tc.tile_pool(name="acts", bufs=1) as apool, \
             tc.tile_pool(name="xin0", bufs=1) as xpool0, \
             tc.tile_pool(name="xinb", bufs=2) as xpoolb, \
             tc.tile_pool(name="work", bufs=2) as work, \
             tc.tile_pool(name="etwork", bufs=7) as etwork, \
             tc.tile_pool(name="ogp", bufs=1) as ogpool, \
             tc.tile_pool(name="stats", bufs=4) as stats, \
             tc.tile_pool(name="bcp", bufs=2) as bcpool, \
             tc.tile_pool(name="ostp", bufs=1) as ostpool, \
             tc.tile_pool(name="psmm", bufs=4, space="PSUM") as ps_mm, \
             tc.tile_pool(name="pspv", bufs=3, space="PSUM") as ps_pv, \
             tc.tile_pool(name="psden", bufs=1, space="PSUM") as ps_den, \
             tc.tile_pool(name="dram", bufs=1, space="DRAM") as dpool:

            # ---------------- constants
            ones_sb = cpool.tile([128, 1], BF, tag="ones", name="ones")
            nc.gpsimd.memset(ones_sb[:], 1.0)
            ones_f = cpool.tile([1, 16], F32, tag="ones_f", name="ones_f")
            nc.gpsimd.memset(ones_f[:], 1.0)
            # warm up the collective path early (gpsimd-issued input DMA so
            # it doesn't queue behind the sync-engine load stream)
            warm_in = dpool.tile([128, 8], BF, tag="warm_in", name="warm_in")
            warm_out = dpool.tile([4, 128, 8], BF, tag="warm_out",
                                  name="warm_out")
            nc.gpsimd.dma_start(out=warm_in[:], in_=cosq[0:128, 0:8])
            nc.gpsimd.collective_compute(
                "AllGather", mybir.AluOpType.bypass,
                replica_groups=groups,
                ins=[warm_in.opt()], outs=[warm_out.opt()])
            # prime the sync/scalar DMA queues: the first DMA on a queue pays
            # ~7-10us of cold-start, so make it a tiny one
            qwarm = cpool.tile([1, 64], BF, tag="qwarm", name="qwarm")
            nc.sync.dma_start(out=qwarm[:, 0:32], in_=cosq[0:1, 0:32])
            nc.scalar.dma_start(out=qwarm[:, 32:64], in_=cosq[0:1, 32:64])

            # ---------------- weights + first x chunk (load order = use order)
            wq_sb = wpool.tile([128, KT, HD_CORE], BF, tag="wq", name="wq")
            wk_sb = wpool.tile([128, KT, D], BF, tag="wk", name="wk")
            wv_sb = wpool.tile([128, KT, D], BF, tag="wv", name="wv")
            wo_sb = wpool.tile([128, KT, HD_CORE], BF, tag="wo", name="wo")

            def load_xc(c):
                if c == 0:
                    # four-batch first chunk so matmul kt can start as soon
                    # as its batch lands
                    t = xpoolb.tile([128, KT, 512], BF, tag="xc0", name="xc0",
                                    bufs=1)
                    for g in range(4):
                        nc.sync.dma_start(out=t[:, ds(4 * g, 4), :],
                                          in_=xt_v[:, ds(4 * g, 4), 0, :])
                    return lambda kt: t[:, kt, :]
                # chunks 1-3: one batched DMA on the scalar queue (the Act
                # engine is lightly loaded during projections)
                t = xpoolb.tile([128, KT, 512], BF, tag="xcb", name="xcb")
                nc.scalar.dma_start(out=t[:], in_=xt_v[:, :, c, :])
                return lambda kt: t[:, kt, :]

            # weights/tables paced to first use: wk then the x batches on
            # sync; wq per-head on scalar so q-head h can start as soon as
            # its slice lands
            nc.sync.dma_start(out=wk_sb[:], in_=wk_v[:])
            xc_state = [load_xc(0)]
            for h in range(G):
                nc.scalar.dma_start(out=wq_sb[:, :, ts(h, 128)],
                                    in_=wq_v[:, :, ts(h, 128)])
            tbl = {}
            for name, src in (("cosk", cosk), ("sink", sink),
                              ("cosq", cosq), ("sinq", sinq)):
                t = cpool.tile([D, S], BF, tag=name)
                nc.gpsimd.dma_start(out=t[:], in_=src[:])
                tbl[name] = t
            nc.sync.dma_start(out=wv_sb[:], in_=wv_v[:])

            # ---------------- persistent activations
            qt_sb = [apool.tile([D, S], BF, tag=f"qt{h}", name=f"qt{h}")
                     for h in range(G)]
            kt_sb = apool.tile([D, S], BF, tag="kt", name="kt")
            vtok_sb = apool.tile([128, TOKB, D], BF, tag="vtok", name="vtok")

            # band et tiles: one buffer per band offset; the [:off] zeros are
            # written once here and never dirtied (exp writes only [off:])
            for b in range(1, 4):
                etb = etwork.tile([128, 512], BF, tag=f"etb{b}",
                                  name=f"etb{b}", bufs=1)
                nc.gpsimd.memset(etb[:, :128 * b], 0.0)
            # warm the gpsimd partition_broadcast ucode library now — the
            # first use otherwise triggers a ~9us LOAD_LIB mid-attention
            warm_bc = bcpool.tile([128, 16], F32, tag="warm_bc",
                                  name="warm_bc")
            nc.gpsimd.partition_broadcast(warm_bc[:], ones_f[:])

            # ---------------- projections + RoPE + v transpose
            def rope_store(raw_ps, c, dst_slice, cos_t, sin_t):
                raw = work.tile([128, 512], BF, tag="qraw", name="qraw")
                nc.scalar.copy(raw[:], raw_ps[:])
                sh = work.tile([128, 512], BF, tag="sh", name="sh")
                nc.sync.dma_start(out=sh[0:64, :], in_=raw[64:128, :])
                nc.sync.dma_start(out=sh[64:128, :], in_=raw[0:64, :])
                t1 = work.tile([128, 512], BF, tag="t1", name="t1")
                nc.vector.tensor_mul(t1[:], sh[:], sin_t[:, ds(512 * c, 512)])
                t2 = work.tile([128, 512], BF, tag="t2", name="t2")
                nc.vector.tensor_mul(t2[:], raw[:], cos_t[:, ds(512 * c, 512)])
                nc.vector.tensor_add(dst_slice, t1[:], t2[:])

            def emit_proj(c):
                xc = xc_state.pop(0)
                if c + 1 < TCH:
                    xc_state.append(load_xc(c + 1))
                # k
                ps = ps_mm.tile([128, 512], F32, tag="mm", name="mm")
                for kt in range(KT):
                    nc.tensor.matmul(ps[:], wk_sb[:, kt, :], xc(kt),
                                     start=(kt == 0), stop=(kt == KT - 1))
                rope_store(ps, c, kt_sb[:, ds(512 * c, 512)],
                           tbl["cosk"], tbl["sink"])
                # q heads
                for h in range(G):
                    ps = ps_mm.tile([128, 512], F32, tag="mm", name="mm")
                    for kt in range(KT):
                        nc.tensor.matmul(ps[:], wq_sb[:, kt, ts(h, 128)],
                                         xc(kt),
                                         start=(kt == 0), stop=(kt == KT - 1))
                    rope_store(ps, c, qt_sb[h][:, ds(512 * c, 512)],
                               tbl["cosq"], tbl["sinq"])
                # v last (no rope; xbar-transpose to token-major)
                ps = ps_mm.tile([128, 512], F32, tag="mm", name="mm")
                for kt in range(KT):
                    nc.tensor.matmul(ps[:], wv_sb[:, kt, :], xc(kt),
                                     start=(kt == 0), stop=(kt == KT - 1))
                vst = work.tile([128, 512], BF, tag="vst", name="vst")
                nc.scalar.copy(vst[:], ps[:])
                for j in range(4):
                    nc.sync.dma_start_transpose(
                        out=vtok_sb[:, 4 * c + j, :], in_=vst[:, ts(j, 128)])

            # ---------------- attention, scores computed pre-transposed
            # each quarter t gathers in two head-pair collectives (issued
            # after h1 and h3) so the Wo input is ready well before wo_mm(t);
            # quarter 3's second pair is further split into two single-head
            # collectives to shrink the end tail.
            cin_p = [[dpool.tile([D, 2, 512], BF, tag=f"cin{t}_{pr}",
                                 name=f"cin{t}_{pr}") for pr in range(2)]
                     for t in range(3)]
            cout_p = [[dpool.tile([4, D, 2, 512], BF, tag=f"cout{t}_{pr}",
                                  name=f"cout{t}_{pr}") for pr in range(2)]
                      for t in range(3)]
            cin_q3a = dpool.tile([D, 2, 512], BF, tag="cinq3a", name="cinq3a")
            cout_q3a = dpool.tile([4, D, 2, 512], BF, tag="coutq3a",
                                  name="coutq3a")
            cin_q3s = [dpool.tile([D, 512], BF, tag=f"cinq3s{g}",
                                  name=f"cinq3s{g}") for g in range(2)]
            cout_q3s = [dpool.tile([4, D, 512], BF, tag=f"coutq3s{g}",
                                   name=f"coutq3s{g}") for g in range(2)]

            def ag(cin, cout):
                nc.gpsimd.collective_compute(
                    "AllGather", mybir.AluOpType.bypass,
                    replica_groups=groups,
                    ins=[cin.opt()], outs=[cout.opt()])

            def wo_load(t):
                """One batched DMA per gathered head-pair buffer.
                Returns list of (kt, og_tile, j) mappings."""
                srcs = []
                if t < 3:
                    for pr in range(2):
                        og = ogpool.tile([128, 4, 2, 512], BF, tag=f"og{pr}",
                                         name=f"og{pr}")
                        nc.sync.dma_start(
                            out=og[:],
                            in_=cout_p[t][pr][:].rearrange(
                                "r p h s -> p r h s"))
                        for r in range(4):
                            for hh in range(2):
                                srcs.append((r * G + 2 * pr + hh, og,
                                             (r, hh)))
                else:
                    # split per source rank across two queues so the first
                    # Wo chains start as soon as the first slices land
                    og = ogpool.tile([128, 4, 2, 512], BF, tag="og0",
                                     name="og0")
                    cv = cout_q3a[:].rearrange("r p h s -> p r h s")
                    for r in range(4):
                        eng = nc.sync if r % 2 == 0 else nc.scalar
                        eng.dma_start(out=og[:, r, :, :], in_=cv[:, r, :, :])
                    for r in range(4):
                        for hh in range(2):
                            srcs.append((r * G + hh, og, (r, hh)))
                    for g in range(2):
                        # both on gpsimd: nothing queues behind them there,
                        # so the q3s1 wait can't block the og0 partial loads
                        ogs = ogpool.tile([128, 4, 512], BF, tag=f"og3s{g}",
                                          name=f"og3s{g}")
                        nc.gpsimd.dma_start(
                            out=ogs[:],
                            in_=cout_q3s[g][:].rearrange("r p s -> p r s"))
                        for r in range(4):
                            srcs.append((r * G + 2 + g, ogs, (r,)))
                srcs.sort()
                return srcs

            def wo_mm(t, srcs):
                # order chains by data-arrival and phase the accumulation so
                # the PE starts on already-gathered head pairs while later
                # collectives are still in flight
                if t == 3:
                    srcs = sorted(srcs, key=lambda kv: (min(kv[0] % G, 2),
                                                        kv[0]))
                    splits = [8, 12, 16]
                elif t >= 1:
                    srcs = sorted(srcs, key=lambda kv: (kv[0] % G >= 2,
                                                        kv[0]))
                    splits = [8, 16]
                else:
                    splits = [16]
                ost = ostpool.tile([128, 4, 512], F32, tag="ost", name="ost")
                pws = [ps_mm.tile([128, 512], F32, tag="mm", name="mm")
                       for _ in range(4)]
                lo = 0
                for hi in splits:
                    for tb in range(4):
                        for idx in range(lo, hi):
                            kt, og, j = srcs[idx]
                            nc.tensor.matmul(
                                pws[tb][(slice(None), slice(None))],
                                og[(slice(None), *j, ts(tb, 128))],
                                wo_sb[:, kt, :],
                                start=(idx == 0), stop=(idx == 15))
                    lo = hi
                for tb in range(4):
                    if t == 3:
                        nc.scalar.copy(ost[:, tb, :], pws[tb][:])
                    else:
                        nc.vector.tensor_copy(out=ost[:, tb, :], in_=pws[tb][:])
                nc.sync.dma_start(out=out_v[:, t, :, :], in_=ost[:])

            def emit_st(h, qc, kb):
                """score block, transposed: [k 128, q<=512] -> exp -> et.
                Causal masking: exp the raw block, then zero the upper
                triangle of the diagonal 128-slab on gpsimd."""
                band = kb - 4 * qc
                if band >= 1:
                    et = etwork.tile([128, 512], BF, tag=f"etb{band}",
                                     name=f"etb{band}", bufs=1)
                else:
                    et = etwork.tile([128, 512], BF, tag="et", name="et")
                sps = ps_mm.tile([128, 512], F32, tag="mm", name="mm")
                if band >= 0:
                    off = 128 * band
                    w = 512 - off
                    nc.tensor.matmul(sps[:, :w], kt_sb[:, ts(kb, 128)],
                                     qt_sb[h][:, ds(512 * qc + off, w)],
                                     start=True, stop=True)
                    nc.scalar.activation(
                        out=et[:, ds(off, w)], in_=sps[:, :w],
                        func=mybir.ActivationFunctionType.Exp)
                    # keep [k_row p, q_col j] iff j >= p within the slab
                    nc.gpsimd.affine_select(
                        out=et[:, ds(off, 128)], in_=et[:, ds(off, 128)],
                        compare_op=mybir.AluOpType.is_ge, fill=0.0,
                        base=0, pattern=[[1, 128]], channel_multiplier=-1)
                    return et, off
                nc.tensor.matmul(sps[:], kt_sb[:, ts(kb, 128)],
                                 qt_sb[h][:, ds(512 * qc, 512)],
                                 start=True, stop=True)
                nc.scalar.activation(
                    out=et[:], in_=sps[:],
                    func=mybir.ActivationFunctionType.Exp)
                return et, 0

            wo_pend = {}
            loads_at = {(2, 0): 0, (2, 3): 1, (3, 1): 2}
            mms_at = {(2, 1): 0, (3, 0): 1, (3, 2): 2}

            def emit_attn(qc):
                for h in range(G):
                    if (qc, h) in loads_at:
                        t = loads_at[(qc, h)]
                        wo_pend[t] = wo_load(t)
                    if (qc, h) in mms_at:
                        t = mms_at[(qc, h)]
                        wo_mm(t, wo_pend.pop(t))
                    nkb = 4 * qc + 4
                    oT_ps = ps_pv.tile([128, 512], F32, tag="pv", name="pv")
                    den_ps = ps_den.tile([1, 512], F32, tag="den", name="den")
                    pend = [emit_st(h, qc, k) for k in range(min(3, nkb))]
                    ngrp = (nkb + 3) // 4
                    esum = None
                    for kb in range(nkb):
                        et, off = pend.pop(0)
                        if kb + 3 < nkb:
                            pend.append(emit_st(h, qc, kb + 3))
                        nc.tensor.matmul(oT_ps[:, ds(off, 512 - off)],
                                         vtok_sb[:, kb, :],
                                         et[:, ds(off, 512 - off)],
                                         start=(kb == 0), stop=(kb == nkb - 1))
                        # denominator: one running DVE sum of the et tiles,
                        # a single ones-matmul at the end of the head
                        if kb == 0:
                            esum = et
                        else:
                            nsum = etwork.tile([128, 512], BF, tag="esum",
                                               name="esum", bufs=3)
                            nc.vector.tensor_add(nsum[:], esum[:], et[:])
                            esum = nsum
                        if kb == nkb - 1:
                            nc.tensor.matmul(den_ps[:], ones_sb[:, 0:1],
                                             esum[:],
                                             start=True, stop=True)
                    rec = stats.tile([1, 512], F32, tag="recq", name="recq")
                    nc.vector.reciprocal_approx_fast(out=rec[:],
                                                     in_=den_ps[:])
                    bcast = bcpool.tile([128, 512], F32, tag="bcast",
                                        name="bcast")
                    nc.gpsimd.partition_broadcast(bcast[:], rec[:])
                    otst = work.tile([128, 512], BF, tag="otst", name="otst")
                    nc.vector.tensor_mul(otst[:], oT_ps[:], bcast[:])
                    if qc < 3:
                        nc.sync.dma_start(out=cin_p[qc][h // 2][:, h % 2, :],
                                          in_=otst[:])
                        if h % 2 == 1:
                            ag(cin_p[qc][h // 2], cout_p[qc][h // 2])
                    elif h < 2:
                        nc.sync.dma_start(out=cin_q3a[:, h, :], in_=otst[:])
                        if h == 1:
                            ag(cin_q3a, cout_q3a)
                    else:
                        nc.sync.dma_start(out=cin_q3s[h - 2][:], in_=otst[:])
                        ag(cin_q3s[h - 2], cout_q3s[h - 2])

            emit_proj(0)
            emit_proj(1)
            emit_attn(0)
            emit_proj(2)
            # wo weights: needed from the first Wo quarter
            nc.sync.dma_start(out=wo_sb[:], in_=wo_v[:])
            emit_attn(1)
            emit_proj(3)
            emit_attn(2)
            emit_attn(3)
            wo_mm(3, wo_load(3))

    nc.finalize()
    return nc


_NC_CACHE = {}


def _get_nc():
    if "nc" not in _NC_CACHE:
        _NC_CACHE["nc"] = build_nc()
    return _NC_CACHE["nc"]


def _rope_tables():
    inv = 1.0 / (THETA ** (np.arange(0, D, 2, dtype=np.float64) / D))  # [64]
    pos = np.arange(S, dtype=np.float64)
    fr = pos[:, None] * inv[None, :]                 # [S, 64]
    emb = np.concatenate([fr, fr], axis=1)           # [S, D]
    cos = np.cos(emb).T.astype(np.float32)           # [D, S]
    sin = np.sin(emb).T.astype(np.float32)
    sgn = np.where(np.arange(D) < D // 2, -1.0, 1.0).astype(np.float32)[:, None]
    scale = np.float32(D ** -0.5)
    return (cos * scale, sin * sgn * scale,          # q tables (pre-scaled)
            cos.copy(), sin * sgn)                   # k tables


def kernel(x, Wq, Wk, Wv, Wo):
    global LAST_EXEC_TIME_NS, LAST_RESULTS
    nc = _get_nc()
    cq, sq, ck, sk = _rope_tables()
    in_maps = []
    for c in range(N_CORES):
        b, kh = c // 4, c % 4
        in_maps.append({
            "xt": np.ascontiguousarray(x[b].T).astype(BF16),
            "wq": np.ascontiguousarray(Wq[:, kh * HD_CORE:(kh + 1) * HD_CORE]).astype(BF16),
            "wk": np.ascontiguousarray(Wk[:, kh * D:(kh + 1) * D]).astype(BF16),
            "wv": np.ascontiguousarray(Wv[:, kh * D:(kh + 1) * D]).astype(BF16),
            "wo": np.ascontiguousarray(Wo[:, kh * HD_CORE:(kh + 1) * HD_CORE]).astype(BF16),
            "cosq": cq.astype(BF16), "sinq": sq.astype(BF16),
            "cosk": ck.astype(BF16), "sink": sk.astype(BF16),
        })
    trace = os.environ.get("KERNEL_TRACE", "0") == "1" and _install_ntff_hook()
    res = run_bass_kernel_spmd(nc, in_maps, core_ids=list(range(N_CORES)),
                               trace=trace)
    LAST_EXEC_TIME_NS = res.exec_time_ns
    LAST_RESULTS = res
    out = np.empty((B, S, DM), dtype=np.float32)
    for c in range(N_CORES):
        b, kh = c // 4, c % 4
        out[b, :, kh * HD_CORE:(kh + 1) * HD_CORE] = res.results[c]["out"]
    return out


# revision 46
# speedup vs baseline: 1.1158x; 1.1158x over previous
"""Distributed Trainium2 kernel for causal GQA attention with RoPE.

Model: B=2, S=2048, DM=2048, H=16 q-heads, HK=4 kv-heads, D=128.
Sharding over 8 NeuronCores: core c = (batch b=c//4, kv-head kh=c%4).
Each core computes its 4 q-heads / 1 kv-head of one batch end-to-end,
AllGathers attention outputs within its 4-core batch group (split into
head-pair collectives for overlap), and applies a column slice of Wo,
producing out[b][:, kh*512:(kh+1)*512].
"""
import contextlib
import ctypes
import os
import sys
import types

for _p in ("/opt/trn_rl_repo", "/root/.axon_site/_ro/trn_rl_repo"):
    if os.path.isdir(_p) and _p not in sys.path:
        sys.path.insert(0, _p)

import numpy as np
import ml_dtypes

import concourse.bass as bass
import concourse.mybir as mybir
import concourse.tile as tile
from concourse import bacc
from concourse.bass import ts, ds
from concourse.bass_utils import run_bass_kernel_spmd
from concourse.masks import make_identity

BF16 = ml_dtypes.bfloat16
F32 = mybir.dt.float32
BF = mybir.dt.bfloat16

B, S, DM = 2, 2048, 2048
H, HK, D = 16, 4, 128
G = H // HK          # q heads per kv head (= heads per core)
THETA = 10000.0
N_CORES = 8
KT = DM // 128       # 16 K-tiles of the model dim
TOKB = S // 128      # 16 token blocks
TCH = S // 512       # 4 token chunks of 512
HD_CORE = G * D      # 512 output dims of q per core

LAST_EXEC_TIME_NS = None
LAST_RESULTS = None


# ---------------------------------------------------------------- tracing
def _install_ntff_hook():
    """Make run_bass_kernel_spmd(trace=True) work in this container."""
    try:
        from antenv.axon_hooks import get_axon_ntff_profile_hook  # noqa: F401
        return True
    except ImportError:
        pass
    so_path = "/opt/axon/libaxon_pjrt.so"
    if not os.path.exists(so_path):
        return False
    lib = ctypes.CDLL(so_path)
    if not hasattr(lib, "axon_start_nrt_profile"):
        return False
    lib.axon_start_nrt_profile.argtypes = [ctypes.POINTER(ctypes.c_int64), ctypes.c_size_t]
    lib.axon_start_nrt_profile.restype = ctypes.c_int64
    lib.axon_stop_nrt_profile.argtypes = [ctypes.c_char_p]
    lib.axon_stop_nrt_profile.restype = ctypes.c_int64

    @contextlib.contextmanager
    def _hook(output_dir, device_ids):
        import jax
        jax.devices()
        if device_ids:
            ids = (ctypes.c_int64 * len(device_ids))(*device_ids)
            rc = lib.axon_start_nrt_profile(ids, len(device_ids))
        else:
            rc = lib.axon_start_nrt_profile(None, 0)
        if rc != 0:
            raise RuntimeError(f"axon_start_nrt_profile rc={rc}")
        try:
            yield
        finally:
            n = lib.axon_stop_nrt_profile(str(output_dir).encode())
            print(f"profile: {n} file(s) in {output_dir}", file=sys.stderr)

    mod = types.ModuleType("antenv.axon_hooks")
    holder = {"h": _hook}
    mod.set_axon_ntff_profile_hook = lambda h: holder.__setitem__("h", h)
    mod.get_axon_ntff_profile_hook = lambda: holder.get("h")
    sys.modules["antenv.axon_hooks"] = mod
    import antenv
    antenv.axon_hooks = mod
    import concourse.bass_utils as bu
    bu.upload_artifacts = lambda tmpdir: str(tmpdir)
    return True


# ---------------------------------------------------------------- graph
def build_nc():
    nc = bacc.Bacc("TRN2", target_bir_lowering=False, debug=False,
                   num_devices=N_CORES)

    xt = nc.dram_tensor("xt", [DM, S], BF, kind="ExternalInput").ap()
    wq = nc.dram_tensor("wq", [DM, HD_CORE], BF, kind="ExternalInput").ap()
    wk = nc.dram_tensor("wk", [DM, D], BF, kind="ExternalInput").ap()
    wv = nc.dram_tensor("wv", [DM, D], BF, kind="ExternalInput").ap()
    wo = nc.dram_tensor("wo", [DM, HD_CORE], BF, kind="ExternalInput").ap()
    cosq = nc.dram_tensor("cosq", [D, S], BF, kind="ExternalInput").ap()
    sinq = nc.dram_tensor("sinq", [D, S], BF, kind="ExternalInput").ap()
    cosk = nc.dram_tensor("cosk", [D, S], BF, kind="ExternalInput").ap()
    sink = nc.dram_tensor("sink", [D, S], BF, kind="ExternalInput").ap()
    out = nc.dram_tensor("out", [S, HD_CORE], F32, kind="ExternalOutput").ap()

    # batched-DMA views (partition-major over the 128-row K-tiles)
    xt_v = xt.rearrange("(kt p) (c s) -> p kt c s", p=128, c=TCH)
    wq_v = wq.rearrange("(kt p) m -> p kt m", p=128)
    wk_v = wk.rearrange("(kt p) m -> p kt m", p=128)
    wv_v = wv.rearrange("(kt p) m -> p kt m", p=128)
    wo_v = wo.rearrange("(kt p) m -> p kt m", p=128)
    out_v = out.rearrange("(t tb p) c -> p t tb c", tb=4, p=128)

    groups = [[0, 1, 2, 3], [4, 5, 6, 7]]

    with tile.TileContext(nc) as tc:
        with tc.tile_pool(name="const", bufs=1) as cpool, \
             tc.tile_pool(name="wts", bufs=1) as wpool, \
             tc.tile_pool(name="acts", bufs=1) as apool, \
             tc.tile_pool(name="xin0", bufs=1) as xpool0, \
             tc.tile_pool(name="xinb", bufs=2) as xpoolb, \
             tc.tile_pool(name="work", bufs=2) as work, \
             tc.tile_pool(name="etwork", bufs=7) as etwork, \
             tc.tile_pool(name="ogp", bufs=1) as ogpool, \
             tc.tile_pool(name="stats", bufs=4) as stats, \
             tc.tile_pool(name="bcp", bufs=2) as bcpool, \
             tc.tile_pool(name="ostp", bufs=1) as ostpool, \
             tc.tile_pool(name="psmm", bufs=4, space="PSUM") as ps_mm, \
             tc.tile_pool(name="pspv", bufs=2, space="PSUM") as ps_pv, \
             tc.tile_pool(name="psden", bufs=1, space="PSUM") as ps_den, \
             tc.tile_pool(name="dram", bufs=1, space="DRAM") as dpool:

            # ---------------- constants
            ident = cpool.tile([128, 128], BF, tag="ident", name="ident")
            make_identity(nc, ident[:])
            ones_sb = cpool.tile([128, 1], BF, tag="ones", name="ones")
            nc.gpsimd.memset(ones_sb[:], 1.0)
            ones_f = cpool.tile([1, 16], F32, tag="ones_f", name="ones_f")
            nc.gpsimd.memset(ones_f[:], 1.0)
            # warm up the collective path early (gpsimd-issued input DMA so
            # it doesn't queue behind the sync-engine load stream)
            warm_in = dpool.tile([128, 8], BF, tag="warm_in", name="warm_in")
            warm_out = dpool.tile([4, 128, 8], BF, tag="warm_out",
                                  name="warm_out")
            nc.gpsimd.dma_start(out=warm_in[:], in_=cosq[0:128, 0:8])
            nc.gpsimd.collective_compute(
                "AllGather", mybir.AluOpType.bypass,
                replica_groups=groups,
                ins=[warm_in.opt()], outs=[warm_out.opt()])
            # prime the sync/scalar DMA queues: the first DMA on a queue pays
            # ~7-10us of cold-start, so make it a tiny one
            qwarm = cpool.tile([1, 64], BF, tag="qwarm", name="qwarm")
            nc.sync.dma_start(out=qwarm[:, 0:32], in_=cosq[0:1, 0:32])
            nc.scalar.dma_start(out=qwarm[:, 32:64], in_=cosq[0:1, 32:64])

            # ---------------- weights + first x chunk (load order = use order)
            wq_sb = wpool.tile([128, KT, HD_CORE], BF, tag="wq", name="wq")
            wk_sb = wpool.tile([128, KT, D], BF, tag="wk", name="wk")
            wv_sb = wpool.tile([128, KT, D], BF, tag="wv", name="wv")
            wo_sb = wpool.tile([128, KT, HD_CORE], BF, tag="wo", name="wo")

            def load_xc(c):
                if c == 0:
                    # four-batch first chunk so matmul kt can start as soon
                    # as its batch lands
                    t = xpoolb.tile([128, KT, 512], BF, tag="xc0", name="xc0",
                                    bufs=1)
                    for g in range(4):
                        nc.sync.dma_start(out=t[:, ds(4 * g, 4), :],
                                          in_=xt_v[:, ds(4 * g, 4), 0, :])
                    return lambda kt: t[:, kt, :]
                # chunks 1-3: one batched DMA on the scalar queue (the Act
                # engine is lightly loaded during projections)
                t = xpoolb.tile([128, KT, 512], BF, tag="xcb", name="xcb")
                nc.scalar.dma_start(out=t[:], in_=xt_v[:, :, c, :])
                return lambda kt: t[:, kt, :]

            # weights/tables paced to first use: wk then the x batches on
            # sync; wq per-head on scalar so q-head h can start as soon as
            # its slice lands
            nc.sync.dma_start(out=wk_sb[:], in_=wk_v[:])
            xc_state = [load_xc(0)]
            for h in range(G):
                nc.scalar.dma_start(out=wq_sb[:, :, ts(h, 128)],
                                    in_=wq_v[:, :, ts(h, 128)])
            # rope tables per-chunk just-in-time (gpsimd queue) to cut the
            # startup bandwidth crunch; chunk 0 quarters now, rest at each
            # emit_proj
            tbl = {}
            tbl_src = {"cosk": cosk, "sink": sink, "cosq": cosq, "sinq": sinq}
            for name, src in tbl_src.items():
                t = cpool.tile([D, S], BF, tag=name, name=name)
                nc.gpsimd.dma_start(out=t[:, 0:512], in_=src[0:D, 0:512])
                tbl[name] = t
            nc.scalar.dma_start(out=wv_sb[:], in_=wv_v[:])

            def load_tables(c):
                for name, src in tbl_src.items():
                    nc.gpsimd.dma_start(
                        out=tbl[name][:, ds(512 * c, 512)],
                        in_=src[0:D, ds(512 * c, 512)])

            # ---------------- persistent activations
            qt_sb = [apool.tile([D, S], BF, tag=f"qt{h}", name=f"qt{h}")
                     for h in range(G)]
            kt_sb = apool.tile([D, S], BF, tag="kt", name="kt")
            vtok_sb = apool.tile([128, TOKB, D], BF, tag="vtok", name="vtok")

            # band et tiles: one buffer per band offset; the [:off] zeros are
            # written once here and never dirtied (exp writes only [off:])
            for b in range(1, 4):
                etb = etwork.tile([128, 512], BF, tag=f"etb{b}",
                                  name=f"etb{b}", bufs=1)
                nc.gpsimd.memset(etb[:, :128 * b], 0.0)
            # warm the gpsimd partition_broadcast ucode library now — the
            # first use otherwise triggers a ~9us LOAD_LIB mid-attention
            warm_bc = bcpool.tile([128, 16], F32, tag="warm_bc",
                                  name="warm_bc")
            nc.gpsimd.partition_broadcast(warm_bc[:], ones_f[:])

            # ---------------- projections + RoPE + v transpose
            def rope_store(raw_ps, c, dst_slice, cos_t, sin_t):
                raw = work.tile([128, 512], BF, tag="qraw", name="qraw")
                nc.scalar.copy(raw[:], raw_ps[:])
                sh = work.tile([128, 512], BF, tag="sh", name="sh")
                nc.sync.dma_start(out=sh[0:64, :], in_=raw[64:128, :])
                nc.sync.dma_start(out=sh[64:128, :], in_=raw[0:64, :])
                t1 = work.tile([128, 512], BF, tag="t1", name="t1")
                nc.vector.tensor_mul(t1[:], sh[:], sin_t[:, ds(512 * c, 512)])
                t2 = work.tile([128, 512], BF, tag="t2", name="t2")
                nc.vector.tensor_mul(t2[:], raw[:], cos_t[:, ds(512 * c, 512)])
                nc.vector.tensor_add(dst_slice, t1[:], t2[:])

            def emit_proj(c):
                xc = xc_state.pop(0)
                if c + 1 < TCH:
                    xc_state.append(load_xc(c + 1))
                if c + 1 < TCH:
                    load_tables(c + 1)
                # k
                ps = ps_mm.tile([128, 512], F32, tag="mm", name="mm")
                for kt in range(KT):
                    nc.tensor.matmul(ps[:], wk_sb[:, kt, :], xc(kt),
                                     start=(kt == 0), stop=(kt == KT - 1))
                rope_store(ps, c, kt_sb[:, ds(512 * c, 512)],
                           tbl["cosk"], tbl["sink"])
                # q heads
                for h in range(G):
                    ps = ps_mm.tile([128, 512], F32, tag="mm", name="mm")
                    for kt in range(KT):
                        nc.tensor.matmul(ps[:], wq_sb[:, kt, ts(h, 128)],
                                         xc(kt),
                                         start=(kt == 0), stop=(kt == KT - 1))
                    rope_store(ps, c, qt_sb[h][:, ds(512 * c, 512)],
                               tbl["cosq"], tbl["sinq"])
                # v last (no rope; xbar-transpose to token-major)
                ps = ps_mm.tile([128, 512], F32, tag="mm", name="mm")
                for kt in range(KT):
                    nc.tensor.matmul(ps[:], wv_sb[:, kt, :], xc(kt),
                                     start=(kt == 0), stop=(kt == KT - 1))
                vst = work.tile([128, 512], BF, tag="vst", name="vst")
                nc.scalar.copy(vst[:], ps[:])
                trp = ps_pv.tile([128, 512], BF, tag="tr", name="trv",
                                 bufs=1)
                for j in range(4):
                    nc.tensor.transpose(trp[:, ts(j, 128)], vst[:, ts(j, 128)],
                                        ident[:])
                nc.vector.tensor_copy(out=vtok_sb[:, ds(4 * c, 4), :],
                                      in_=trp[:])

            # ---------------- attention, scores computed pre-transposed
            # each quarter t gathers in two head-pair collectives (issued
            # after h1 and h3) so the Wo input is ready well before wo_mm(t);
            # quarter 3's second pair is further split into two single-head
            # collectives to shrink the end tail.
            cin_p = [[dpool.tile([D, 2, 512], BF, tag=f"cin{t}_{pr}",
                                 name=f"cin{t}_{pr}") for pr in range(2)]
                     for t in range(3)]
            cout_p = [[dpool.tile([4, D, 2, 512], BF, tag=f"cout{t}_{pr}",
                                  name=f"cout{t}_{pr}") for pr in range(2)]
                      for t in range(3)]
            cin_q3a = dpool.tile([D, 2, 512], BF, tag="cinq3a", name="cinq3a")
            cout_q3a = dpool.tile([4, D, 2, 512], BF, tag="coutq3a",
                                  name="coutq3a")
            cin_q3s = [dpool.tile([D, 512], BF, tag=f"cinq3s{g}",
                                  name=f"cinq3s{g}") for g in range(2)]
            cout_q3s = [dpool.tile([4, D, 512], BF, tag=f"coutq3s{g}",
                                   name=f"coutq3s{g}") for g in range(2)]

            def ag(cin, cout):
                nc.gpsimd.collective_compute(
                    "AllGather", mybir.AluOpType.bypass,
                    replica_groups=groups,
                    ins=[cin.opt()], outs=[cout.opt()])

            def wo_load(t):
                """One batched DMA per gathered head-pair buffer.
                Returns list of (kt, og_tile, j) mappings."""
                srcs = []
                if t < 3:
                    for pr in range(2):
                        og = ogpool.tile([128, 4, 2, 512], BF, tag=f"og{pr}",
                                         name=f"og{pr}")
                        nc.sync.dma_start(
                            out=og[:],
                            in_=cout_p[t][pr][:].rearrange(
                                "r p h s -> p r h s"))
                        for r in range(4):
                            for hh in range(2):
                                srcs.append((r * G + 2 * pr + hh, og,
                                             (r, hh)))
                else:
                    # split per source rank across two queues so the first
                    # Wo chains start as soon as the first slices land
                    og = ogpool.tile([128, 4, 2, 512], BF, tag="og0",
                                     name="og0")
                    cv = cout_q3a[:].rearrange("r p h s -> p r h s")
                    for r in range(4):
                        eng = nc.sync if r % 2 == 0 else nc.scalar
                        eng.dma_start(out=og[:, r, :, :], in_=cv[:, r, :, :])
                    for r in range(4):
                        for hh in range(2):
                            srcs.append((r * G + hh, og, (r, hh)))
                    for g in range(2):
                        # both on gpsimd: nothing queues behind them there,
                        # so the q3s1 wait can't block the og0 partial loads
                        ogs = ogpool.tile([128, 4, 512], BF, tag=f"og3s{g}",
                                          name=f"og3s{g}")
                        nc.gpsimd.dma_start(
                            out=ogs[:],
                            in_=cout_q3s[g][:].rearrange("r p s -> p r s"))
                        for r in range(4):
                            srcs.append((r * G + 2 + g, ogs, (r,)))
                srcs.sort()
                return srcs

            def wo_mm(t, srcs):
                # order chains by data-arrival and phase the accumulation so
                # the PE starts on already-gathered head pairs while later
                # collectives are still in flight
                if t == 3:
                    srcs = sorted(srcs, key=lambda kv: (min(kv[0] % G, 2),
                                                        kv[0]))
                    splits = [8, 12, 16]
                elif t >= 1:
                    srcs = sorted(srcs, key=lambda kv: (kv[0] % G >= 2,
                                                        kv[0]))
                    splits = [8, 16]
                else:
                    splits = [16]
                ost = ostpool.tile([128, 4, 512], F32, tag="ost", name="ost")
                pws = [ps_mm.tile([128, 512], F32, tag="mm", name="mm")
                       for _ in range(4)]
                lo = 0
                for hi in splits:
                    for tb in range(4):
                        for idx in range(lo, hi):
                            kt, og, j = srcs[idx]
                            nc.tensor.matmul(
                                pws[tb][(slice(None), slice(None))],
                                og[(slice(None), *j, ts(tb, 128))],
                                wo_sb[:, kt, :],
                                start=(idx == 0), stop=(idx == 15))
                    lo = hi
                for tb in range(4):
                    if t == 3:
                        # tail: copy on Act and store each block immediately
                        nc.scalar.copy(ost[:, tb, :], pws[tb][:])
                        nc.sync.dma_start(out=out_v[:, t, tb, :],
                                          in_=ost[:, tb, :])
                    else:
                        nc.vector.tensor_copy(out=ost[:, tb, :], in_=pws[tb][:])
                if t < 3:
                    nc.sync.dma_start(out=out_v[:, t, :, :], in_=ost[:])

            def emit_st(h, qc, kb):
                """score block, transposed: [k 128, q<=512] -> exp -> et.
                Causal masking: exp the raw block, then zero the upper
                triangle of the diagonal 128-slab on gpsimd."""
                band = kb - 4 * qc
                if band >= 1:
                    et = etwork.tile([128, 512], BF, tag=f"etb{band}",
                                     name=f"etb{band}", bufs=1)
                else:
                    et = etwork.tile([128, 512], BF, tag="et", name="et")
                sps = ps_mm.tile([128, 512], F32, tag="mm", name="mm")
                if band >= 0:
                    off = 128 * band
                    w = 512 - off
                    nc.tensor.matmul(sps[:, :w], kt_sb[:, ts(kb, 128)],
                                     qt_sb[h][:, ds(512 * qc + off, w)],
                                     start=True, stop=True)
                    nc.scalar.activation(
                        out=et[:, ds(off, w)], in_=sps[:, :w],
                        func=mybir.ActivationFunctionType.Exp)
                    # keep [k_row p, q_col j] iff j >= p within the slab
                    nc.gpsimd.affine_select(
                        out=et[:, ds(off, 128)], in_=et[:, ds(off, 128)],
                        compare_op=mybir.AluOpType.is_ge, fill=0.0,
                        base=0, pattern=[[1, 128]], channel_multiplier=-1)
                    return et, off
                nc.tensor.matmul(sps[:], kt_sb[:, ts(kb, 128)],
                                 qt_sb[h][:, ds(512 * qc, 512)],
                                 start=True, stop=True)
                nc.scalar.activation(
                    out=et[:], in_=sps[:],
                    func=mybir.ActivationFunctionType.Exp)
                return et, 0

            wo_pend = {}
            loads_at = {(2, 0): 0, (2, 3): 1, (3, 1): 2}
            mms_at = {(2, 1): 0, (3, 0): 1, (3, 2): 2}

            def emit_attn(qc):
                for h in range(G):
                    if (qc, h) in loads_at:
                        t = loads_at[(qc, h)]
                        wo_pend[t] = wo_load(t)
                    if (qc, h) in mms_at:
                        t = mms_at[(qc, h)]
                        wo_mm(t, wo_pend.pop(t))
                    nkb = 4 * qc + 4
                    oT_ps = ps_pv.tile([128, 512], F32, tag="pv", name="pv")
                    den_ps = ps_den.tile([1, 512], F32, tag="den", name="den")
                    pend = [emit_st(h, qc, k) for k in range(min(3, nkb))]
                    ngrp = (nkb + 3) // 4
                    esum = None
                    for kb in range(nkb):
                        et, off = pend.pop(0)
                        if kb + 3 < nkb:
                            pend.append(emit_st(h, qc, kb + 3))
                        nc.tensor.matmul(oT_ps[:, ds(off, 512 - off)],
                                         vtok_sb[:, kb, :],
                                         et[:, ds(off, 512 - off)],
                                         start=(kb == 0), stop=(kb == nkb - 1))
                        # denominator: one running DVE sum of the et tiles,
                        # a single ones-matmul at the end of the head
                        if kb == 0:
                            esum = et
                        else:
                            nsum = etwork.tile([128, 512], BF, tag="esum",
                                               name="esum", bufs=3)
                            nc.vector.tensor_add(nsum[:], esum[:], et[:])
                            esum = nsum
                        if kb == nkb - 1:
                            nc.tensor.matmul(den_ps[:], ones_sb[:, 0:1],
                                             esum[:],
                                             start=True, stop=True)
                    # release the PV PSUM bank early (ps_pv ring is 2 deep)
                    oT_sb = work.tile([128, 512], BF, tag="oT_sb",
                                      name="oT_sb")
                    nc.vector.tensor_copy(out=oT_sb[:], in_=oT_ps[:])
                    rec = stats.tile([1, 512], F32, tag="recq", name="recq")
                    nc.vector.reciprocal_approx_fast(out=rec[:],
                                                     in_=den_ps[:])
                    bcast = bcpool.tile([128, 512], F32, tag="bcast",
                                        name="bcast")
                    nc.gpsimd.partition_broadcast(bcast[:], rec[:])
                    otst = work.tile([128, 512], BF, tag="otst", name="otst")
                    nc.vector.tensor_mul(otst[:], oT_sb[:], bcast[:])
                    if qc < 3:
                        nc.sync.dma_start(out=cin_p[qc][h // 2][:, h % 2, :],
                                          in_=otst[:])
                        if h % 2 == 1:
                            ag(cin_p[qc][h // 2], cout_p[qc][h // 2])
                    elif h < 2:
                        nc.sync.dma_start(out=cin_q3a[:, h, :], in_=otst[:])
                        if h == 1:
                            ag(cin_q3a, cout_q3a)
                    else:
                        nc.sync.dma_start(out=cin_q3s[h - 2][:], in_=otst[:])
                        ag(cin_q3s[h - 2], cout_q3s[h - 2])

            emit_proj(0)
            emit_proj(1)
            emit_attn(0)
            emit_proj(2)
            # wo weights: needed from the first Wo quarter
            nc.sync.dma_start(out=wo_sb[:], in_=wo_v[:])
            emit_attn(1)
            emit_proj(3)
            emit_attn(2)
            emit_attn(3)
            wo_mm(3, wo_load(3))

    nc.finalize()
    return nc


_NC_CACHE = {}


def _get_nc():
    if "nc" not in _NC_CACHE:
        _NC_CACHE["nc"] = build_nc()
    return _NC_CACHE["nc"]


def _rope_tables():
    inv = 1.0 / (THETA ** (np.arange(0, D, 2, dtype=np.float64) / D))  # [64]
    pos = np.arange(S, dtype=np.float64)
    fr = pos[:, None] * inv[None, :]                 # [S, 64]
    emb = np.concatenate([fr, fr], axis=1)           # [S, D]
    cos = np.cos(emb).T.astype(np.float32)           # [D, S]
    sin = np.sin(emb).T.astype(np.float32)
    sgn = np.where(np.arange(D) < D // 2, -1.0, 1.0).astype(np.float32)[:, None]
    scale = np.float32(D ** -0.5)
    return (cos * scale, sin * sgn * scale,          # q tables (pre-scaled)
            cos.copy(), sin * sgn)                   # k tables


def kernel(x, Wq, Wk, Wv, Wo):
    global LAST_EXEC_TIME_NS, LAST_RESULTS
    nc = _get_nc()
    cq, sq, ck, sk = _rope_tables()
    in_maps = []
    for c in range(N_CORES):
        b, kh = c // 4, c % 4
        in_maps.append({
            "xt": np.ascontiguousarray(x[b].T).astype(BF16),
            "wq": np.ascontiguousarray(Wq[:, kh * HD_CORE:(kh + 1) * HD_CORE]).astype(BF16),
            "wk": np.ascontiguousarray(Wk[:, kh * D:(kh + 1) * D]).astype(BF16),
            "wv": np.ascontiguousarray(Wv[:, kh * D:(kh + 1) * D]).astype(BF16),
            "wo": np.ascontiguousarray(Wo[:, kh * HD_CORE:(kh + 1) * HD_CORE]).astype(BF16),
            "cosq": cq.astype(BF16), "sinq": sq.astype(BF16),
            "cosk": ck.astype(BF16), "sink": sk.astype(BF16),
        })
    trace = os.environ.get("KERNEL_TRACE", "0") == "1" and _install_ntff_hook()
    res = run_bass_kernel_spmd(nc, in_maps, core_ids=list(range(N_CORES)),
                               trace=trace)
    LAST_EXEC_TIME_NS = res.exec_time_ns
    LAST_RESULTS = res
    out = np.empty((B, S, DM), dtype=np.float32)
    for c in range(N_CORES):
        b, kh = c // 4, c % 4
        out[b, :, kh * HD_CORE:(kh + 1) * HD_CORE] = res.results[c]["out"]
    return out
